# revision 4
# baseline (speedup 1.0000x reference)
"""DeepSeek-V3 MoE block on 8 Trainium2 NeuronCores (Bass/Tile).

Sharding strategy (expert-parallel + shared-expert TP):
  - 32 routed experts -> 4 per core (expert-parallel). Every core receives the
    full token set (512 tokens, replicated) so no token dispatch is needed;
    each core computes its 4 experts' contribution for all tokens densely
    (route weights are 0 for non-selected (token, expert) pairs, so the dense
    weighted sum is mathematically identical to top-k routing).
  - Shared expert: TP-shard the 2048-wide intermediate -> 256 per core.
  - Gate + routing: replicated on every core (tiny), computed in fp32.
  - Each core produces a partial [512, 2048] output (its experts + its shared
    shard); an on-device ReduceScatter sums partials and leaves rows
    [64c:64c+64] on core c; the host concatenates the 8 slices.

Layout/dtype choices:
  - Expert + shared matmuls run in bf16 (PE runs fp32 at 1/4 throughput; bf16
    also halves the ~100MB/core weight traffic). Accumulation is fp32 in PSUM.
  - Gate matmul and all routing math stay fp32 (top-8 boundary margins in this
    problem go down to ~4e-5; bf16 there would flip expert selections).
  - All weights are pre-transposed on the host so the contraction dim lands on
    the SBUF partition axis with contiguous DMA lines.
  - Activations are produced directly in [inter, token] layout so the down
    projection needs no transposes; route weights are applied by broadcasting
    route_w rows across partitions with a one-hot selector matmul.
"""

import numpy as np
import ml_dtypes

import concourse.bass as bass
import concourse.mybir as mybir
import concourse.tile as tile
from concourse import bacc
from concourse.bass import ds, ts
from concourse.masks import make_identity

F32 = mybir.dt.float32
BF16 = mybir.dt.bfloat16
AF = mybir.ActivationFunctionType
ALU = mybir.AluOpType

N_CORES = 8
T = 512          # tokens
H = 2048         # hidden
I = 1024         # routed expert intermediate
E = 32           # total experts
E_LOC = E // N_CORES   # 4 experts per core
SH = 2048 // N_CORES   # shared-expert intermediate shard per core
KT = H // 128    # 16 contraction tiles
TT = T // 128    # 4 token tiles
IT = I // 128    # 8 inter tiles per expert
SHT = SH // 128  # 2 shared inter tiles
ROUT_SCALE = 2.5
T_OUT = T // N_CORES   # 64 rows of final output per core


def build_nc():
    nc = bacc.Bacc("TRN2", target_bir_lowering=False, debug=False,
                   num_devices=N_CORES)

    # ---- I/O ----
    xt_bf = nc.dram_tensor("xt_bf", [H, T], BF16, kind="ExternalInput")
    xt_f32 = nc.dram_tensor("xt_f32", [H, T], F32, kind="ExternalInput")
    gwt = nc.dram_tensor("gwt", [H, E], F32, kind="ExternalInput")
    gbias = nc.dram_tensor("gbias", [1, E], F32, kind="ExternalInput")
    selb = nc.dram_tensor("selb", [E_LOC, E, 128], F32, kind="ExternalInput")
    # w13t[e, 0] = ex_w1[global_e].T (H x I);  w13t[e, 1] = ex_w3[global_e].T
    w13t = nc.dram_tensor("w13t", [E_LOC, 2, H, I], BF16, kind="ExternalInput")
    w2t = nc.dram_tensor("w2t", [E_LOC, I, H], BF16, kind="ExternalInput")
    # shgut[0] = sh_wg.T slice (H x SH), shgut[1] = sh_wu.T slice
    shgut = nc.dram_tensor("shgut", [2, H, SH], BF16, kind="ExternalInput")
    shwdt = nc.dram_tensor("shwdt", [SH, H], BF16, kind="ExternalInput")
    y = nc.dram_tensor("y", [T_OUT, H], F32, kind="ExternalOutput")
    rw_dbg = nc.dram_tensor("rw_dbg", [T, E], F32, kind="ExternalOutput")

    # internal DRAM for the collective (collectives can't touch I/O tensors)
    cc_in = nc.dram_tensor("cc_in", [T, H], F32)
    cc_out = nc.dram_tensor("cc_out", [T_OUT, H], F32)

    with tile.TileContext(nc) as tc:
        with (
            tc.tile_pool(name="const", bufs=1) as cpool,
            tc.tile_pool(name="xbf", bufs=KT) as xbf_pool,
            tc.tile_pool(name="xf32", bufs=2) as xf32_pool,
            tc.tile_pool(name="rout", bufs=1) as rpool,
            tc.tile_pool(name="rscr", bufs=2) as rscr,
            tc.tile_pool(name="wstream", bufs=8) as wpool,
            tc.tile_pool(name="w2stream", bufs=10) as w2pool,
            tc.tile_pool(name="silu", bufs=5) as spool,
            tc.tile_pool(name="tmp", bufs=2) as tpool,
            tc.tile_pool(name="actsh", bufs=2) as shpool,
            tc.tile_pool(name="actw", bufs=E_LOC * IT) as awpool,
            tc.tile_pool(name="acc", bufs=TT) as accpool,
            tc.tile_pool(name="psum_gu", bufs=4, space="PSUM") as pgu,
            tc.tile_pool(name="psum_dn", bufs=2, space="PSUM") as pdn,
            tc.tile_pool(name="psum_sm", bufs=2, space="PSUM") as psm,
        ):
            # ---------- constants ----------
            ident = cpool.tile([128, 128], F32, tag="ident")
            make_identity(nc, ident[:])

            bias_sb = cpool.tile([128, E], F32, tag="bias")
            nc.sync.dma_start(bias_sb[:], gbias.ap().partition_broadcast(128))

            selb_sb = cpool.tile([E, E_LOC, 128], F32, tag="selb")
            nc.sync.dma_start(selb_sb[:], selb.ap().rearrange("j e i -> e j i"))

            gwt_sb = cpool.tile([128, KT, E], F32, tag="gwt")
            nc.sync.dma_start(gwt_sb[:],
                              gwt.ap().rearrange("(kt p) e -> p kt e", p=128))

            # ---------- resident xT (bf16) ----------
            xbf = []
            for k in range(KT):
                t_ = xbf_pool.tile([128, T], BF16, tag="xbf")
                nc.sync.dma_start(t_[:], xt_bf.ap()[ts(k, 128), :])
                xbf.append(t_)

            # ---------- gate matmul (fp32): logitsT [E, T] ----------
            logT_ps = psm.tile([E, T], F32, tag="sm")
            for k in range(KT):
                xf = xf32_pool.tile([128, T], F32, tag="xf")
                nc.sync.dma_start(xf[:], xt_f32.ap()[ts(k, 128), :])
                nc.tensor.matmul(logT_ps[:], gwt_sb[:, k, :], xf[:],
                                 start=(k == 0), stop=(k == KT - 1))
            logT_sb = rpool.tile([E, T], F32, tag="logT")
            nc.scalar.copy(logT_sb[:], logT_ps[:])

            # ---------- routing (fp32, per 128-token tile) ----------
            route_w = rpool.tile([128, TT, E], F32, tag="routew")
            scr = rpool.tile([128, 14 * 8], F32, tag="scr")
            for t in range(TT):
                # transpose logits tile back to [128 tokens, 32 experts]
                lg_ps = psm.tile([128, E], F32, tag="sm")
                nc.tensor.transpose(lg_ps[:], logT_sb[:, ts(t, 128)],
                                    ident[:E, :E])
                scores = rscr.tile([128, E], F32, tag="scores")
                nc.scalar.activation(scores[:], lg_ps[:], AF.Sigmoid)
                swb = rscr.tile([128, E], F32, tag="swb")
                nc.vector.tensor_add(swb[:], scores[:], bias_sb[:])

                # group scores: sum of top-2 of each group of 4 =
                # max over the 6 pairwise sums within the group
                swb_g = swb[:].rearrange("p (g u) -> p g u", u=4)

                def sv(idx):
                    return scr[:, ds(idx * 8, 8)]

                pairs = [(0, 1), (2, 3), (0, 2), (1, 3), (0, 3), (1, 2)]
                for n, (a, b) in enumerate(pairs):
                    nc.vector.tensor_add(sv(n), swb_g[:, :, a], swb_g[:, :, b])
                nc.vector.tensor_max(sv(6), sv(0), sv(1))
                nc.vector.tensor_max(sv(7), sv(2), sv(3))
                nc.vector.tensor_max(sv(8), sv(4), sv(5))
                nc.vector.tensor_max(sv(9), sv(6), sv(7))
                nc.vector.tensor_max(sv(10), sv(8), sv(9))  # group scores

                # top-4 groups -> mask
                g8 = sv(11)
                nc.vector.max(g8, sv(10))
                gmask = sv(12)
                nc.vector.tensor_scalar(gmask, sv(10), g8[:, 3:4], None,
                                        op0=ALU.is_ge)
                # expand to experts and mask scores+bias
                swbm = rscr.tile([128, E], F32, tag="swbm")
                nc.vector.tensor_tensor(
                    out=swbm[:].rearrange("p (g u) -> p g u", u=4),
                    in0=swb_g,
                    in1=gmask.to_broadcast([128, 8, 4]),
                    op=ALU.mult)
                # top-8 experts of masked scores
                e8 = sv(13)
                nc.vector.max(e8, swbm[:])
                emask = rscr.tile([128, E], F32, tag="emask")
                nc.vector.tensor_scalar(emask[:], swbm[:], e8[:, 7:8], None,
                                        op0=ALU.is_ge)
                sel = rscr.tile([128, E], F32, tag="sel")
                nc.vector.tensor_mul(sel[:], scores[:], emask[:])
                den = rscr.tile([128, 2], F32, tag="den")
                nc.vector.reduce_sum(den[:, 0:1], sel[:],
                                     axis=mybir.AxisListType.X)
                nc.vector.tensor_scalar_add(den[:, 0:1], den[:, 0:1], 1e-20)
                nc.vector.reciprocal(den[:, 1:2], den[:, 0:1])
                nc.vector.tensor_scalar(route_w[:, t, :], sel[:], den[:, 1:2],
                                        ROUT_SCALE, op0=ALU.mult, op1=ALU.mult)
                nc.sync.dma_start(rw_dbg.ap()[ts(t, 128), :], route_w[:, t, :])

            # ---------- route_w -> rwT [E, T] -> per-expert partition bcast --
            rwT = rpool.tile([E, T], F32, tag="rwT")
            for t in range(TT):
                rw_ps = psm.tile([E, 128], F32, tag="sm")
                nc.tensor.transpose(rw_ps[:], route_w[:, t, :], ident[:])
                nc.scalar.copy(rwT[:, ts(t, 128)], rw_ps[:])

            # rwb_sb[:, j, t] = route_w[t, 4c+j] replicated across partitions
            rwb_sb = rpool.tile([128, E_LOC, T], F32, tag="rwb")
            for j in range(E_LOC):
                b_ps = pdn.tile([128, T], F32, tag="dn")
                nc.tensor.matmul(b_ps[:], selb_sb[:, j, :], rwT[:],
                                 start=True, stop=True)
                nc.scalar.copy(rwb_sb[:, j, :], b_ps[:])

            # ---------- shared expert up/gate: actT_sh [SH, T] (bf16) -------
            actsh = []
            for s in range(SHT):
                ps_g = pgu.tile([128, T], F32, tag="gu")
                for k in range(KT):
                    wg = wpool.tile([128, 128], BF16, tag="wsh")
                    nc.sync.dma_start(wg[:],
                                      shgut.ap()[0, ts(k, 128), ts(s, 128)])
                    nc.tensor.matmul(ps_g[:], wg[:], xbf[k][:],
                                     start=(k == 0), stop=(k == KT - 1))
                sg = spool.tile([128, T], F32, tag="silu")
                nc.scalar.activation(sg[:], ps_g[:], AF.Silu)
                ps_u = pgu.tile([128, T], F32, tag="gu")
                for k in range(KT):
                    wu = wpool.tile([128, 128], BF16, tag="wsh")
                    nc.sync.dma_start(wu[:],
                                      shgut.ap()[1, ts(k, 128), ts(s, 128)])
                    nc.tensor.matmul(ps_u[:], wu[:], xbf[k][:],
                                     start=(k == 0), stop=(k == KT - 1))
                a_ = shpool.tile([128, T], BF16, tag="actsh")
                nc.vector.tensor_mul(a_[:], sg[:], ps_u[:])
                actsh.append(a_)

            # ---------- routed experts up/gate -> actw[j][i] [128, T] bf16 --
            actw = [[None] * IT for _ in range(E_LOC)]
            for j in range(E_LOC):
                for half in range(2):
                    # gate proj (w1) for this i-half
                    ps_gs = []
                    for k in range(KT):
                        wch = wpool.tile([128, 512], BF16, tag="w13")
                        nc.sync.dma_start(
                            wch[:], w13t.ap()[j, 0, ts(k, 128),
                                              ds(half * 512, 512)])
                        for ii in range(4):
                            if k == 0:
                                ps_gs.append(pgu.tile([128, T], F32, tag="gu", name="psg"))
                            nc.tensor.matmul(ps_gs[ii][:], wch[:, ts(ii, 128)],
                                             xbf[k][:], start=(k == 0),
                                             stop=(k == KT - 1))
                    sgs = []
                    for ii in range(4):
                        sg = spool.tile([128, T], F32, tag="silu")
                        nc.scalar.activation(sg[:], ps_gs[ii][:], AF.Silu)
                        sgs.append(sg)
                    # up proj (w3) for this i-half
                    ps_us = []
                    for k in range(KT):
                        wch = wpool.tile([128, 512], BF16, tag="w13")
                        nc.sync.dma_start(
                            wch[:], w13t.ap()[j, 1, ts(k, 128),
                                              ds(half * 512, 512)])
                        for ii in range(4):
                            if k == 0:
                                ps_us.append(pgu.tile([128, T], F32, tag="gu", name="psu"))
                            nc.tensor.matmul(ps_us[ii][:], wch[:, ts(ii, 128)],
                                             xbf[k][:], start=(k == 0),
                                             stop=(k == KT - 1))
                    for ii in range(4):
                        i_ = half * 4 + ii
                        tmp = tpool.tile([128, T], F32, tag="tmp")
                        nc.vector.tensor_mul(tmp[:], sgs[ii][:], ps_us[ii][:])
                        aw = awpool.tile([128, T], BF16, tag="actw")
                        nc.vector.tensor_mul(aw[:], tmp[:], rwb_sb[:, j, :])
                        actw[j][i_] = aw

            # ---------- down projection + fp32 accumulation ----------
            # acc[t] [128, H] f32; shared expert first (init by copy), then
            # the 4 routed experts (accumulate-adds).
            acc = [accpool.tile([128, H], F32, tag="acc", name=f"acc{t}")
                   for t in range(TT)]

            shd = []
            for s in range(SHT):
                c_ = w2pool.tile([128, H], BF16, tag="w2")
                nc.sync.dma_start(c_[:], shwdt.ap()[ts(s, 128), :])
                shd.append(c_)
            for t in range(TT):
                for h in range(4):
                    ps_d = pdn.tile([128, 512], F32, tag="dn")
                    for s in range(SHT):
                        nc.tensor.matmul(ps_d[:],
                                         actsh[s][:, ts(t, 128)],
                                         shd[s][:, ds(h * 512, 512)],
                                         start=(s == 0), stop=(s == SHT - 1))
                    nc.scalar.copy(acc[t][:, ds(h * 512, 512)], ps_d[:])

            for j in range(E_LOC):
                w2ch = []
                for i_ in range(IT):
                    c_ = w2pool.tile([128, H], BF16, tag="w2")
                    nc.sync.dma_start(c_[:], w2t.ap()[j, ts(i_, 128), :])
                    w2ch.append(c_)
                for t in range(TT):
                    for h in range(4):
                        ps_d = pdn.tile([128, 512], F32, tag="dn")
                        for i_ in range(IT):
                            nc.tensor.matmul(ps_d[:],
                                             actw[j][i_][:, ts(t, 128)],
                                             w2ch[i_][:, ds(h * 512, 512)],
                                             start=(i_ == 0),
                                             stop=(i_ == IT - 1))
                        sl = acc[t][:, ds(h * 512, 512)]
                        nc.vector.tensor_add(sl, sl, ps_d[:])

            # ---------- combine across cores ----------
            for t in range(TT):
                nc.sync.dma_start(cc_in.ap()[ts(t, 128), :], acc[t][:])
            nc.gpsimd.collective_compute(
                "ReduceScatter", ALU.add,
                replica_groups=[list(range(N_CORES))],
                ins=[cc_in.ap().opt()], outs=[cc_out.ap().opt()])
            nc.sync.dma_start(y.ap(), cc_out.ap())

    nc.compile()
    return nc


# ---------------------------------------------------------------------------
# host-side sharding / layout prep
# ---------------------------------------------------------------------------

def prep_in_maps(hidden_states, gate_w, gate_bias, sh_wg, sh_wu, sh_wd,
                 ex_w1, ex_w3, ex_w2):
    bf = ml_dtypes.bfloat16
    x = np.ascontiguousarray(np.asarray(hidden_states, np.float32))
    xt = np.ascontiguousarray(x.T)                     # [H, T] f32
    xt_bf = xt.astype(bf)
    gwt = np.ascontiguousarray(np.asarray(gate_w, np.float32).T)  # [H, E]
    gb = np.ascontiguousarray(np.asarray(gate_bias, np.float32).reshape(1, E))

    w1t = np.ascontiguousarray(
        np.asarray(ex_w1, np.float32).transpose(0, 2, 1)).astype(bf)  # [E,H,I]
    w3t = np.ascontiguousarray(
        np.asarray(ex_w3, np.float32).transpose(0, 2, 1)).astype(bf)
    w2t_full = np.ascontiguousarray(
        np.asarray(ex_w2, np.float32).transpose(0, 2, 1)).astype(bf)  # [E,I,H]
    shwgt = np.ascontiguousarray(np.asarray(sh_wg, np.float32).T).astype(bf)
    shwut = np.ascontiguousarray(np.asarray(sh_wu, np.float32).T).astype(bf)
    shwdt_full = np.ascontiguousarray(
        np.asarray(sh_wd, np.float32).T).astype(bf)

    in_maps = []
    for c in range(N_CORES):
        sl = slice(c * E_LOC, (c + 1) * E_LOC)
        sel = np.zeros((E_LOC, E, 128), np.float32)
        for j in range(E_LOC):
            sel[j, c * E_LOC + j, :] = 1.0
        w13 = np.stack([w1t[sl], w3t[sl]], axis=1)     # [E_LOC, 2, H, I]
        ssl = slice(c * SH, (c + 1) * SH)
        in_maps.append({
            "xt_bf": xt_bf,
            "xt_f32": xt,
            "gwt": gwt,
            "gbias": gb,
            "selb": sel,
            "w13t": np.ascontiguousarray(w13),
            "w2t": np.ascontiguousarray(w2t_full[sl]),
            "shgut": np.ascontiguousarray(
                np.stack([shwgt[:, ssl], shwut[:, ssl]], axis=0)),
            "shwdt": np.ascontiguousarray(shwdt_full[ssl, :]),
        })
    return in_maps


_CACHE = {}


def get_nc():
    if "nc" not in _CACHE:
        _CACHE["nc"] = build_nc()
    return _CACHE["nc"]


def kernel(**inputs) -> np.ndarray:
    nc = get_nc()
    in_maps = prep_in_maps(**inputs)
    from concourse.bass_utils import run_bass_kernel_spmd
    res = run_bass_kernel_spmd(nc, in_maps, core_ids=list(range(N_CORES)))
    out = np.concatenate([res.results[c]["y"] for c in range(N_CORES)], axis=0)
    return np.ascontiguousarray(out.astype(np.float32))


# revision 9
# speedup vs baseline: 4.4172x; 4.4172x over previous
"""DeepSeek-V3 MoE block on 8 Trainium2 NeuronCores (Bass/Tile).

Sharding strategy (expert-parallel + shared-expert TP):
  - 32 routed experts -> 4 per core (expert-parallel). Every core receives the
    full token set (512 tokens, replicated) so no token dispatch is needed;
    each core computes its 4 experts' contribution for all tokens densely
    (route weights are 0 for non-selected (token, expert) pairs, so the dense
    weighted sum is mathematically identical to top-k routing).
  - Shared expert: TP-shard the 2048-wide intermediate -> 256 per core.
  - Gate + routing: replicated on every core (tiny), computed in fp32.
  - Each core produces a partial [512, 2048] output (its experts + its shared
    shard); an on-device ReduceScatter sums partials and leaves rows
    [64c:64c+64] on core c; the host concatenates the 8 slices.

Layout/dtype choices:
  - Expert + shared matmuls run in bf16 (PE runs fp32 at 1/4 throughput; bf16
    also halves the ~100MB/core weight traffic). Accumulation is fp32 in PSUM.
  - Gate matmul and all routing math stay fp32 (top-8 boundary margins in this
    problem go down to ~4e-5; bf16 there would flip expert selections).
  - All weights are pre-transposed on the host so the contraction dim lands on
    the SBUF partition axis with contiguous DMA lines.
  - Activations are produced directly in [inter, token] layout so the down
    projection needs no transposes; route weights are applied by broadcasting
    route_w rows across partitions with a one-hot selector matmul.
"""

import numpy as np
import ml_dtypes

import concourse.bass as bass
import concourse.mybir as mybir
import concourse.tile as tile
from concourse import bacc
from concourse.bass import ds, ts
from concourse.masks import make_identity

F32 = mybir.dt.float32
BF16 = mybir.dt.bfloat16
AF = mybir.ActivationFunctionType
ALU = mybir.AluOpType

N_CORES = 8
T = 512          # tokens
H = 2048         # hidden
I = 1024         # routed expert intermediate
E = 32           # total experts
E_LOC = E // N_CORES   # 4 experts per core
SH = 2048 // N_CORES   # shared-expert intermediate shard per core
KT = H // 128    # 16 contraction tiles
TT = T // 128    # 4 token tiles
IT = I // 128    # 8 inter tiles per expert
SHT = SH // 128  # 2 shared inter tiles
ROUT_SCALE = 2.5
T_OUT = T // N_CORES   # 64 rows of final output per core


def build_nc(reps: int = 1):
    nc = bacc.Bacc("TRN2", target_bir_lowering=False, debug=False,
                   num_devices=N_CORES)

    # ---- I/O ----
    xt_bf = nc.dram_tensor("xt_bf", [H, T], BF16, kind="ExternalInput")
    xt_f32 = nc.dram_tensor("xt_f32", [H, T], F32, kind="ExternalInput")
    gwt = nc.dram_tensor("gwt", [H, E], F32, kind="ExternalInput")
    gbias = nc.dram_tensor("gbias", [1, E], F32, kind="ExternalInput")
    selb = nc.dram_tensor("selb", [E_LOC, E, 128], F32, kind="ExternalInput")
    # w13t[e, 0] = ex_w1[global_e].T (H x I);  w13t[e, 1] = ex_w3[global_e].T
    w13t = nc.dram_tensor("w13t", [E_LOC, 2, H, I], BF16, kind="ExternalInput")
    w2t = nc.dram_tensor("w2t", [E_LOC, I, H], BF16, kind="ExternalInput")
    # shgut[0] = sh_wg.T slice (H x SH), shgut[1] = sh_wu.T slice
    shgut = nc.dram_tensor("shgut", [2, H, SH], BF16, kind="ExternalInput")
    shwdt = nc.dram_tensor("shwdt", [SH, H], BF16, kind="ExternalInput")
    y = nc.dram_tensor("y", [T_OUT, H], F32, kind="ExternalOutput")
    rw_dbg = nc.dram_tensor("rw_dbg", [T, E], F32, kind="ExternalOutput")

    # internal DRAM for the collective (collectives can't touch I/O tensors)
    cc_in = nc.dram_tensor("cc_in", [T, H], F32)
    cc_out = nc.dram_tensor("cc_out", [T_OUT, H], F32)

    with tile.TileContext(nc) as tc:
        with (
            tc.tile_pool(name="const", bufs=1) as cpool,
            tc.tile_pool(name="xbf", bufs=KT) as xbf_pool,
            tc.tile_pool(name="xf32", bufs=2) as xf32_pool,
            tc.tile_pool(name="rout", bufs=1) as rpool,
            tc.tile_pool(name="rscr", bufs=2) as rscr,
            tc.tile_pool(name="wstream", bufs=8) as wpool,
            tc.tile_pool(name="w2stream", bufs=10) as w2pool,
            tc.tile_pool(name="silu", bufs=5) as spool,
            tc.tile_pool(name="tmp", bufs=2) as tpool,
            tc.tile_pool(name="actsh", bufs=2) as shpool,
            tc.tile_pool(name="actw", bufs=IT + 6) as awpool,
            tc.tile_pool(name="acc", bufs=TT) as accpool,
            tc.tile_pool(name="psum_gu", bufs=5, space="PSUM") as pgu,
            tc.tile_pool(name="psum_dn", bufs=2, space="PSUM") as pdn,
            tc.tile_pool(name="psum_sm", bufs=1, space="PSUM") as psm,
        ):
          for _rep in range(reps):
            # ---------- constants ----------
            ident = cpool.tile([128, 128], F32, tag="ident")
            make_identity(nc, ident[:])

            bias_sb = cpool.tile([128, E], F32, tag="bias")
            nc.sync.dma_start(bias_sb[:], gbias.ap().partition_broadcast(128))

            selb_sb = cpool.tile([E, E_LOC, 128], F32, tag="selb")
            nc.sync.dma_start(selb_sb[:], selb.ap().rearrange("j e i -> e j i"))

            gwt_sb = cpool.tile([128, KT, E], F32, tag="gwt")
            nc.sync.dma_start(gwt_sb[:],
                              gwt.ap().rearrange("(kt p) e -> p kt e", p=128))

            # ---------- resident xT (bf16) ----------
            xbf = []
            for k in range(KT):
                t_ = xbf_pool.tile([128, T], BF16, tag="xbf")
                nc.sync.dma_start(t_[:], xt_bf.ap()[ts(k, 128), :])
                xbf.append(t_)

            # ---------- gate matmul (fp32): logitsT [E, T] ----------
            logT_ps = psm.tile([E, T], F32, tag="sm")
            for k in range(KT):
                xf = xf32_pool.tile([128, T], F32, tag="xf")
                nc.sync.dma_start(xf[:], xt_f32.ap()[ts(k, 128), :])
                nc.tensor.matmul(logT_ps[:], gwt_sb[:, k, :], xf[:],
                                 start=(k == 0), stop=(k == KT - 1))
            logT_sb = rpool.tile([E, T], F32, tag="logT")
            nc.scalar.copy(logT_sb[:], logT_ps[:])

            # ---------- routing (fp32, per 128-token tile) ----------
            route_w = rpool.tile([128, TT, E], F32, tag="routew")
            scr = rpool.tile([128, 14 * 8], F32, tag="scr")
            for t in range(TT):
                # transpose logits tile back to [128 tokens, 32 experts]
                lg_ps = psm.tile([128, E], F32, tag="sm")
                nc.tensor.transpose(lg_ps[:], logT_sb[:, ts(t, 128)],
                                    ident[:E, :E])
                scores = rscr.tile([128, E], F32, tag="scores")
                nc.scalar.activation(scores[:], lg_ps[:], AF.Sigmoid)
                swb = rscr.tile([128, E], F32, tag="swb")
                nc.vector.tensor_add(swb[:], scores[:], bias_sb[:])

                # group scores: sum of top-2 of each group of 4 =
                # max over the 6 pairwise sums within the group
                swb_g = swb[:].rearrange("p (g u) -> p g u", u=4)

                def sv(idx):
                    return scr[:, ds(idx * 8, 8)]

                pairs = [(0, 1), (2, 3), (0, 2), (1, 3), (0, 3), (1, 2)]
                for n, (a, b) in enumerate(pairs):
                    nc.vector.tensor_add(sv(n), swb_g[:, :, a], swb_g[:, :, b])
                nc.vector.tensor_max(sv(6), sv(0), sv(1))
                nc.vector.tensor_max(sv(7), sv(2), sv(3))
                nc.vector.tensor_max(sv(8), sv(4), sv(5))
                nc.vector.tensor_max(sv(9), sv(6), sv(7))
                nc.vector.tensor_max(sv(10), sv(8), sv(9))  # group scores

                # top-4 groups -> mask
                g8 = sv(11)
                nc.vector.max(g8, sv(10))
                gmask = sv(12)
                nc.vector.tensor_scalar(gmask, sv(10), g8[:, 3:4], None,
                                        op0=ALU.is_ge)
                # expand to experts and mask scores+bias
                swbm = rscr.tile([128, E], F32, tag="swbm")
                nc.vector.tensor_tensor(
                    out=swbm[:].rearrange("p (g u) -> p g u", u=4),
                    in0=swb_g,
                    in1=gmask.to_broadcast([128, 8, 4]),
                    op=ALU.mult)
                # top-8 experts of masked scores
                e8 = sv(13)
                nc.vector.max(e8, swbm[:])
                emask = rscr.tile([128, E], F32, tag="emask")
                nc.vector.tensor_scalar(emask[:], swbm[:], e8[:, 7:8], None,
                                        op0=ALU.is_ge)
                sel = rscr.tile([128, E], F32, tag="sel")
                nc.vector.tensor_mul(sel[:], scores[:], emask[:])
                den = rscr.tile([128, 2], F32, tag="den")
                nc.vector.reduce_sum(den[:, 0:1], sel[:],
                                     axis=mybir.AxisListType.X)
                nc.vector.tensor_scalar_add(den[:, 0:1], den[:, 0:1], 1e-20)
                nc.vector.reciprocal(den[:, 1:2], den[:, 0:1])
                nc.vector.tensor_scalar(route_w[:, t, :], sel[:], den[:, 1:2],
                                        ROUT_SCALE, op0=ALU.mult, op1=ALU.mult)
                nc.sync.dma_start(rw_dbg.ap()[ts(t, 128), :], route_w[:, t, :])

            # ---------- route_w -> rwT [E, T] -> per-expert partition bcast --
            rwT = rpool.tile([E, T], F32, tag="rwT")
            for t in range(TT):
                rw_ps = psm.tile([E, 128], F32, tag="sm")
                nc.tensor.transpose(rw_ps[:], route_w[:, t, :], ident[:])
                nc.scalar.copy(rwT[:, ts(t, 128)], rw_ps[:])

            # rwb_sb[:, j, t] = route_w[t, 4c+j] replicated across partitions
            rwb_sb = rpool.tile([128, E_LOC, T], F32, tag="rwb")
            for j in range(E_LOC):
                b_ps = pdn.tile([128, T], F32, tag="dn")
                nc.tensor.matmul(b_ps[:], selb_sb[:, j, :], rwT[:],
                                 start=True, stop=True)
                nc.scalar.copy(rwb_sb[:, j, :], b_ps[:])

            # ---------- shared expert up/gate: actT_sh [SH, T] (bf16) -------
            actsh = []
            for s in range(SHT):
                ps_g = pgu.tile([128, T], F32, tag="gu")
                for k in range(KT):
                    wg = wpool.tile([128, 128], BF16, tag="wsh")
                    nc.sync.dma_start(wg[:],
                                      shgut.ap()[0, ts(k, 128), ts(s, 128)])
                    nc.tensor.matmul(ps_g[:], wg[:], xbf[k][:],
                                     start=(k == 0), stop=(k == KT - 1))
                sg = spool.tile([128, T], F32, tag="silu")
                nc.scalar.activation(sg[:], ps_g[:], AF.Silu)
                ps_u = pgu.tile([128, T], F32, tag="gu")
                for k in range(KT):
                    wu = wpool.tile([128, 128], BF16, tag="wsh")
                    nc.sync.dma_start(wu[:],
                                      shgut.ap()[1, ts(k, 128), ts(s, 128)])
                    nc.tensor.matmul(ps_u[:], wu[:], xbf[k][:],
                                     start=(k == 0), stop=(k == KT - 1))
                a_ = shpool.tile([128, T], BF16, tag="actsh")
                nc.vector.tensor_mul(a_[:], sg[:], ps_u[:])
                actsh.append(a_)

            # ---------- shared-expert down: initialises acc ----------
            acc = [accpool.tile([128, H], F32, tag="acc", name=f"acc{t}")
                   for t in range(TT)]
            shd = []
            for s in range(SHT):
                c_ = w2pool.tile([128, H], BF16, tag="w2")
                nc.sync.dma_start(c_[:], shwdt.ap()[ts(s, 128), :])
                shd.append(c_)
            for t in range(TT):
                for h in range(4):
                    ps_d = pdn.tile([128, 512], F32, tag="dn")
                    for s in range(SHT):
                        nc.tensor.matmul(ps_d[:],
                                         actsh[s][:, ts(t, 128)],
                                         shd[s][:, ds(h * 512, 512)],
                                         start=(s == 0), stop=(s == SHT - 1))
                    nc.scalar.copy(acc[t][:, ds(h * 512, 512)], ps_d[:])

            # ---------- routed experts up/gate -> actw[j][i] [128, T] bf16 --
            actw = [[None] * IT for _ in range(E_LOC)]
            for j in range(E_LOC):
                for half in range(2):
                    # gate proj (w1) for this i-half
                    ps_gs = []
                    for k in range(KT):
                        wch = wpool.tile([128, 512], BF16, tag="w13")
                        nc.sync.dma_start(
                            wch[:], w13t.ap()[j, 0, ts(k, 128),
                                              ds(half * 512, 512)])
                        for ii in range(4):
                            if k == 0:
                                ps_gs.append(pgu.tile([128, T], F32, tag="gu", name="psg"))
                            nc.tensor.matmul(ps_gs[ii][:], wch[:, ts(ii, 128)],
                                             xbf[k][:], start=(k == 0),
                                             stop=(k == KT - 1))
                    sgs = []
                    for ii in range(4):
                        sg = spool.tile([128, T], F32, tag="silu")
                        nc.scalar.activation(sg[:], ps_gs[ii][:], AF.Silu)
                        sgs.append(sg)
                    # up proj (w3) for this i-half
                    ps_us = []
                    for k in range(KT):
                        wch = wpool.tile([128, 512], BF16, tag="w13")
                        nc.sync.dma_start(
                            wch[:], w13t.ap()[j, 1, ts(k, 128),
                                              ds(half * 512, 512)])
                        for ii in range(4):
                            if k == 0:
                                ps_us.append(pgu.tile([128, T], F32, tag="gu", name="psu"))
                            nc.tensor.matmul(ps_us[ii][:], wch[:, ts(ii, 128)],
                                             xbf[k][:], start=(k == 0),
                                             stop=(k == KT - 1))
                    for ii in range(4):
                        i_ = half * 4 + ii
                        tmp = tpool.tile([128, T], F32, tag="tmp")
                        nc.vector.tensor_mul(tmp[:], sgs[ii][:], ps_us[ii][:])
                        aw = awpool.tile([128, T], BF16, tag="actw")
                        nc.vector.tensor_mul(aw[:], tmp[:], rwb_sb[:, j, :])
                        actw[j][i_] = aw

                # down projection for expert j (accumulate into acc)
                w2ch = []
                for i_ in range(IT):
                    c_ = w2pool.tile([128, H], BF16, tag="w2")
                    nc.sync.dma_start(c_[:], w2t.ap()[j, ts(i_, 128), :])
                    w2ch.append(c_)
                for t in range(TT):
                    for h in range(4):
                        ps_d = pdn.tile([128, 512], F32, tag="dn")
                        for i_ in range(IT):
                            nc.tensor.matmul(ps_d[:],
                                             actw[j][i_][:, ts(t, 128)],
                                             w2ch[i_][:, ds(h * 512, 512)],
                                             start=(i_ == 0),
                                             stop=(i_ == IT - 1))
                        sl = acc[t][:, ds(h * 512, 512)]
                        nc.vector.tensor_add(sl, sl, ps_d[:])

            # ---------- combine across cores ----------
            for t in range(TT):
                nc.sync.dma_start(cc_in.ap()[ts(t, 128), :], acc[t][:])
            nc.gpsimd.collective_compute(
                "ReduceScatter", ALU.add,
                replica_groups=[list(range(N_CORES))],
                ins=[cc_in.ap().opt()], outs=[cc_out.ap().opt()])
            nc.sync.dma_start(y.ap(), cc_out.ap())

    nc.compile()
    return nc


# ---------------------------------------------------------------------------
# host-side sharding / layout prep
# ---------------------------------------------------------------------------

def prep_in_maps(hidden_states, gate_w, gate_bias, sh_wg, sh_wu, sh_wd,
                 ex_w1, ex_w3, ex_w2):
    bf = ml_dtypes.bfloat16
    x = np.ascontiguousarray(np.asarray(hidden_states, np.float32))
    xt = np.ascontiguousarray(x.T)                     # [H, T] f32
    xt_bf = xt.astype(bf)
    gwt = np.ascontiguousarray(np.asarray(gate_w, np.float32).T)  # [H, E]
    gb = np.ascontiguousarray(np.asarray(gate_bias, np.float32).reshape(1, E))

    w1t = np.ascontiguousarray(
        np.asarray(ex_w1, np.float32).transpose(0, 2, 1)).astype(bf)  # [E,H,I]
    w3t = np.ascontiguousarray(
        np.asarray(ex_w3, np.float32).transpose(0, 2, 1)).astype(bf)
    w2t_full = np.ascontiguousarray(
        np.asarray(ex_w2, np.float32).transpose(0, 2, 1)).astype(bf)  # [E,I,H]
    shwgt = np.ascontiguousarray(np.asarray(sh_wg, np.float32).T).astype(bf)
    shwut = np.ascontiguousarray(np.asarray(sh_wu, np.float32).T).astype(bf)
    shwdt_full = np.ascontiguousarray(
        np.asarray(sh_wd, np.float32).T).astype(bf)

    in_maps = []
    for c in range(N_CORES):
        sl = slice(c * E_LOC, (c + 1) * E_LOC)
        sel = np.zeros((E_LOC, E, 128), np.float32)
        for j in range(E_LOC):
            sel[j, c * E_LOC + j, :] = 1.0
        w13 = np.stack([w1t[sl], w3t[sl]], axis=1)     # [E_LOC, 2, H, I]
        ssl = slice(c * SH, (c + 1) * SH)
        in_maps.append({
            "xt_bf": xt_bf,
            "xt_f32": xt,
            "gwt": gwt,
            "gbias": gb,
            "selb": sel,
            "w13t": np.ascontiguousarray(w13),
            "w2t": np.ascontiguousarray(w2t_full[sl]),
            "shgut": np.ascontiguousarray(
                np.stack([shwgt[:, ssl], shwut[:, ssl]], axis=0)),
            "shwdt": np.ascontiguousarray(shwdt_full[ssl, :]),
        })
    return in_maps


_CACHE = {}


def get_nc():
    if "nc" not in _CACHE:
        _CACHE["nc"] = build_nc()
    return _CACHE["nc"]


def kernel(**inputs) -> np.ndarray:
    nc = get_nc()
    in_maps = prep_in_maps(**inputs)
    from concourse.bass_utils import run_bass_kernel_spmd
    res = run_bass_kernel_spmd(nc, in_maps, core_ids=list(range(N_CORES)))
    out = np.concatenate([res.results[c]["y"] for c in range(N_CORES)], axis=0)
    return np.ascontiguousarray(out.astype(np.float32))


# revision 10
# speedup vs baseline: 4.5418x; 1.0282x over previous
"""DeepSeek-V3 MoE block on 8 Trainium2 NeuronCores (Bass/Tile).

Sharding strategy (expert-parallel + shared-expert TP):
  - 32 routed experts -> 4 per core (expert-parallel). Every core receives the
    full token set (512 tokens, replicated) so no token dispatch is needed;
    each core computes its 4 experts' contribution for all tokens densely
    (route weights are 0 for non-selected (token, expert) pairs, so the dense
    weighted sum is mathematically identical to top-k routing).
  - Shared expert: TP-shard the 2048-wide intermediate -> 256 per core.
  - Gate + routing: replicated on every core (tiny), computed in fp32.
  - Each core produces a partial [512, 2048] output (its experts + its shared
    shard); an on-device ReduceScatter sums partials and leaves rows
    [64c:64c+64] on core c; the host concatenates the 8 slices.

Layout/dtype choices:
  - Expert + shared matmuls run in bf16 (PE runs fp32 at 1/4 throughput; bf16
    also halves the ~100MB/core weight traffic). Accumulation is fp32 in PSUM.
  - Gate matmul and all routing math stay fp32 (top-8 boundary margins in this
    problem go down to ~4e-5; bf16 there would flip expert selections).
  - All weights are pre-transposed on the host so the contraction dim lands on
    the SBUF partition axis with contiguous DMA lines.
  - Activations are produced directly in [inter, token] layout so the down
    projection needs no transposes; route weights are applied by broadcasting
    route_w rows across partitions with a one-hot selector matmul.
"""

import numpy as np
import ml_dtypes

import concourse.bass as bass
import concourse.mybir as mybir
import concourse.tile as tile
from concourse import bacc
from concourse.bass import ds, ts
from concourse.masks import make_identity

F32 = mybir.dt.float32
BF16 = mybir.dt.bfloat16
AF = mybir.ActivationFunctionType
ALU = mybir.AluOpType

N_CORES = 8
T = 512          # tokens
H = 2048         # hidden
I = 1024         # routed expert intermediate
E = 32           # total experts
E_LOC = E // N_CORES   # 4 experts per core
SH = 2048 // N_CORES   # shared-expert intermediate shard per core
KT = H // 128    # 16 contraction tiles
TT = T // 128    # 4 token tiles
IT = I // 128    # 8 inter tiles per expert
SHT = SH // 128  # 2 shared inter tiles
ROUT_SCALE = 2.5
T_OUT = T // N_CORES   # 64 rows of final output per core


def build_nc(reps: int = 1):
    nc = bacc.Bacc("TRN2", target_bir_lowering=False, debug=False,
                   num_devices=N_CORES)

    # ---- I/O ----
    xt_bf = nc.dram_tensor("xt_bf", [H, T], BF16, kind="ExternalInput")
    xt_f32 = nc.dram_tensor("xt_f32", [H, T], F32, kind="ExternalInput")
    gwt = nc.dram_tensor("gwt", [H, E], F32, kind="ExternalInput")
    gbias = nc.dram_tensor("gbias", [1, E], F32, kind="ExternalInput")
    selb = nc.dram_tensor("selb", [E_LOC, E, 128], F32, kind="ExternalInput")
    # w13t[e, 0] = ex_w1[global_e].T (H x I);  w13t[e, 1] = ex_w3[global_e].T
    w13t = nc.dram_tensor("w13t", [E_LOC, 2, H, I], BF16, kind="ExternalInput")
    w2t = nc.dram_tensor("w2t", [E_LOC, I, H], BF16, kind="ExternalInput")
    # shgut[0] = sh_wg.T slice (H x SH), shgut[1] = sh_wu.T slice
    shgut = nc.dram_tensor("shgut", [2, H, SH], BF16, kind="ExternalInput")
    shwdt = nc.dram_tensor("shwdt", [SH, H], BF16, kind="ExternalInput")
    y = nc.dram_tensor("y", [T_OUT, H], F32, kind="ExternalOutput")
    rw_dbg = nc.dram_tensor("rw_dbg", [T, E], F32, kind="ExternalOutput")

    # internal DRAM for the collective (collectives can't touch I/O tensors)
    cc_in = nc.dram_tensor("cc_in", [T, H], F32)
    cc_out = nc.dram_tensor("cc_out", [T_OUT, H], F32)

    from contextlib import ExitStack
    with tile.TileContext(nc) as tc:
        with ExitStack() as _st:
            cpool = _st.enter_context(tc.tile_pool(name="const", bufs=1))
            xbf_pool = _st.enter_context(tc.tile_pool(name="xbf", bufs=KT))
            xf32_pool = _st.enter_context(tc.tile_pool(name="xf32", bufs=2))
            rpool = _st.enter_context(tc.tile_pool(name="rout", bufs=1))
            rscr = _st.enter_context(tc.tile_pool(name="rscr", bufs=2))
            wpool = _st.enter_context(tc.tile_pool(name="wstream", bufs=8))
            w2pool = _st.enter_context(tc.tile_pool(name="w2stream", bufs=10))
            spool = _st.enter_context(tc.tile_pool(name="silu", bufs=5))
            tpool = _st.enter_context(tc.tile_pool(name="tmp", bufs=2))
            shpool = _st.enter_context(tc.tile_pool(name="actsh", bufs=2))
            awpool = _st.enter_context(tc.tile_pool(name="actw", bufs=IT + 6))
            accpool = _st.enter_context(tc.tile_pool(name="acc", bufs=TT))
            pgu = _st.enter_context(
                tc.tile_pool(name="psum_gu", bufs=5, space="PSUM"))
            pdn = _st.enter_context(
                tc.tile_pool(name="psum_dn", bufs=2, space="PSUM"))
            psm = _st.enter_context(
                tc.tile_pool(name="psum_sm", bufs=1, space="PSUM"))
            for _rep in range(reps):
              # ---------- constants ----------
              ident = cpool.tile([128, 128], F32, tag="ident")
              make_identity(nc, ident[:])

              bias_sb = cpool.tile([128, E], F32, tag="bias")
              nc.sync.dma_start(bias_sb[:], gbias.ap().partition_broadcast(128))

              selb_sb = cpool.tile([E, E_LOC, 128], F32, tag="selb")
              nc.sync.dma_start(selb_sb[:], selb.ap().rearrange("j e i -> e j i"))

              gwt_sb = cpool.tile([128, KT, E], F32, tag="gwt")
              nc.sync.dma_start(gwt_sb[:],
                                gwt.ap().rearrange("(kt p) e -> p kt e", p=128))

              # ---------- resident xT (bf16) ----------
              xbf = []
              for k in range(KT):
                  t_ = xbf_pool.tile([128, T], BF16, tag="xbf")
                  nc.sync.dma_start(t_[:], xt_bf.ap()[ts(k, 128), :])
                  xbf.append(t_)

              # ---------- gate matmul (fp32): logitsT [E, T] ----------
              logT_ps = psm.tile([E, T], F32, tag="sm")
              for k in range(KT):
                  xf = xf32_pool.tile([128, T], F32, tag="xf")
                  nc.sync.dma_start(xf[:], xt_f32.ap()[ts(k, 128), :])
                  nc.tensor.matmul(logT_ps[:], gwt_sb[:, k, :], xf[:],
                                   start=(k == 0), stop=(k == KT - 1))
              logT_sb = rpool.tile([E, T], F32, tag="logT")
              nc.scalar.copy(logT_sb[:], logT_ps[:])

              # ---------- routing (fp32, per 128-token tile) ----------
              route_w = rpool.tile([128, TT, E], F32, tag="routew")
              scr = rpool.tile([128, 14 * 8], F32, tag="scr")
              for t in range(TT):
                  # transpose logits tile back to [128 tokens, 32 experts]
                  lg_ps = psm.tile([128, E], F32, tag="sm")
                  nc.tensor.transpose(lg_ps[:], logT_sb[:, ts(t, 128)],
                                      ident[:E, :E])
                  scores = rscr.tile([128, E], F32, tag="scores")
                  nc.scalar.activation(scores[:], lg_ps[:], AF.Sigmoid)
                  swb = rscr.tile([128, E], F32, tag="swb")
                  nc.vector.tensor_add(swb[:], scores[:], bias_sb[:])

                  # group scores: sum of top-2 of each group of 4 =
                  # max over the 6 pairwise sums within the group
                  swb_g = swb[:].rearrange("p (g u) -> p g u", u=4)

                  def sv(idx):
                      return scr[:, ds(idx * 8, 8)]

                  pairs = [(0, 1), (2, 3), (0, 2), (1, 3), (0, 3), (1, 2)]
                  for n, (a, b) in enumerate(pairs):
                      nc.vector.tensor_add(sv(n), swb_g[:, :, a], swb_g[:, :, b])
                  nc.vector.tensor_max(sv(6), sv(0), sv(1))
                  nc.vector.tensor_max(sv(7), sv(2), sv(3))
                  nc.vector.tensor_max(sv(8), sv(4), sv(5))
                  nc.vector.tensor_max(sv(9), sv(6), sv(7))
                  nc.vector.tensor_max(sv(10), sv(8), sv(9))  # group scores

                  # top-4 groups -> mask
                  g8 = sv(11)
                  nc.vector.max(g8, sv(10))
                  gmask = sv(12)
                  nc.vector.tensor_scalar(gmask, sv(10), g8[:, 3:4], None,
                                          op0=ALU.is_ge)
                  # expand to experts and mask scores+bias
                  swbm = rscr.tile([128, E], F32, tag="swbm")
                  nc.vector.tensor_tensor(
                      out=swbm[:].rearrange("p (g u) -> p g u", u=4),
                      in0=swb_g,
                      in1=gmask.to_broadcast([128, 8, 4]),
                      op=ALU.mult)
                  # top-8 experts of masked scores
                  e8 = sv(13)
                  nc.vector.max(e8, swbm[:])
                  emask = rscr.tile([128, E], F32, tag="emask")
                  nc.vector.tensor_scalar(emask[:], swbm[:], e8[:, 7:8], None,
                                          op0=ALU.is_ge)
                  sel = rscr.tile([128, E], F32, tag="sel")
                  nc.vector.tensor_mul(sel[:], scores[:], emask[:])
                  den = rscr.tile([128, 2], F32, tag="den")
                  nc.vector.reduce_sum(den[:, 0:1], sel[:],
                                       axis=mybir.AxisListType.X)
                  nc.vector.tensor_scalar_add(den[:, 0:1], den[:, 0:1], 1e-20)
                  nc.vector.reciprocal(den[:, 1:2], den[:, 0:1])
                  nc.vector.tensor_scalar(route_w[:, t, :], sel[:], den[:, 1:2],
                                          ROUT_SCALE, op0=ALU.mult, op1=ALU.mult)
                  nc.sync.dma_start(rw_dbg.ap()[ts(t, 128), :], route_w[:, t, :])

              # ---------- route_w -> rwT [E, T] -> per-expert partition bcast --
              rwT = rpool.tile([E, T], F32, tag="rwT")
              for t in range(TT):
                  rw_ps = psm.tile([E, 128], F32, tag="sm")
                  nc.tensor.transpose(rw_ps[:], route_w[:, t, :], ident[:])
                  nc.scalar.copy(rwT[:, ts(t, 128)], rw_ps[:])

              # rwb_sb[:, j, t] = route_w[t, 4c+j] replicated across partitions
              rwb_sb = rpool.tile([128, E_LOC, T], F32, tag="rwb")
              for j in range(E_LOC):
                  b_ps = pdn.tile([128, T], F32, tag="dn")
                  nc.tensor.matmul(b_ps[:], selb_sb[:, j, :], rwT[:],
                                   start=True, stop=True)
                  nc.scalar.copy(rwb_sb[:, j, :], b_ps[:])

              # ---------- shared expert up/gate: actT_sh [SH, T] (bf16) -------
              actsh = []
              for s in range(SHT):
                  ps_g = pgu.tile([128, T], F32, tag="gu")
                  for k in range(KT):
                      wg = wpool.tile([128, 128], BF16, tag="wsh")
                      nc.sync.dma_start(wg[:],
                                        shgut.ap()[0, ts(k, 128), ts(s, 128)])
                      nc.tensor.matmul(ps_g[:], wg[:], xbf[k][:],
                                       start=(k == 0), stop=(k == KT - 1))
                  sg = spool.tile([128, T], F32, tag="silu")
                  nc.scalar.activation(sg[:], ps_g[:], AF.Silu)
                  ps_u = pgu.tile([128, T], F32, tag="gu")
                  for k in range(KT):
                      wu = wpool.tile([128, 128], BF16, tag="wsh")
                      nc.sync.dma_start(wu[:],
                                        shgut.ap()[1, ts(k, 128), ts(s, 128)])
                      nc.tensor.matmul(ps_u[:], wu[:], xbf[k][:],
                                       start=(k == 0), stop=(k == KT - 1))
                  a_ = shpool.tile([128, T], BF16, tag="actsh")
                  nc.vector.tensor_mul(a_[:], sg[:], ps_u[:])
                  actsh.append(a_)

              # ---------- shared-expert down: initialises acc ----------
              acc = [accpool.tile([128, H], F32, tag="acc", name=f"acc{t}")
                     for t in range(TT)]
              shd = []
              for s in range(SHT):
                  c_ = w2pool.tile([128, H], BF16, tag="w2", name="shd")
                  nc.sync.dma_start(c_[:], shwdt.ap()[ts(s, 128), :])
                  shd.append(c_)
              for t in range(TT):
                  for h in range(4):
                      ps_d = pdn.tile([128, 512], F32, tag="dn")
                      for s in range(SHT):
                          nc.tensor.matmul(ps_d[:],
                                           actsh[s][:, ts(t, 128)],
                                           shd[s][:, ds(h * 512, 512)],
                                           start=(s == 0), stop=(s == SHT - 1))
                      nc.scalar.copy(acc[t][:, ds(h * 512, 512)], ps_d[:])

              # ---------- routed experts up/gate -> actw[j][i] [128, T] bf16 --
              actw = [[None] * IT for _ in range(E_LOC)]
              for j in range(E_LOC):
                  for half in range(2):
                      # gate proj (w1) for this i-half
                      ps_gs = []
                      for k in range(KT):
                          wch = wpool.tile([128, 512], BF16, tag="w13")
                          nc.sync.dma_start(
                              wch[:], w13t.ap()[j, 0, ts(k, 128),
                                                ds(half * 512, 512)])
                          for ii in range(4):
                              if k == 0:
                                  ps_gs.append(pgu.tile([128, T], F32, tag="gu", name="psg"))
                              nc.tensor.matmul(ps_gs[ii][:], wch[:, ts(ii, 128)],
                                               xbf[k][:], start=(k == 0),
                                               stop=(k == KT - 1))
                      sgs = []
                      for ii in range(4):
                          sg = spool.tile([128, T], F32, tag="silu")
                          nc.scalar.activation(sg[:], ps_gs[ii][:], AF.Silu)
                          sgs.append(sg)
                      # up proj (w3) for this i-half
                      ps_us = []
                      for k in range(KT):
                          wch = wpool.tile([128, 512], BF16, tag="w13")
                          nc.sync.dma_start(
                              wch[:], w13t.ap()[j, 1, ts(k, 128),
                                                ds(half * 512, 512)])
                          for ii in range(4):
                              if k == 0:
                                  ps_us.append(pgu.tile([128, T], F32, tag="gu", name="psu"))
                              nc.tensor.matmul(ps_us[ii][:], wch[:, ts(ii, 128)],
                                               xbf[k][:], start=(k == 0),
                                               stop=(k == KT - 1))
                      for ii in range(4):
                          i_ = half * 4 + ii
                          tmp = tpool.tile([128, T], F32, tag="tmp")
                          nc.vector.tensor_mul(tmp[:], sgs[ii][:], ps_us[ii][:])
                          aw = awpool.tile([128, T], BF16, tag="actw")
                          nc.vector.tensor_mul(aw[:], tmp[:], rwb_sb[:, j, :])
                          actw[j][i_] = aw

                  # down projection for expert j (accumulate into acc)
                  w2ch = []
                  for i_ in range(IT):
                      c_ = w2pool.tile([128, H], BF16, tag="w2", name="w2c")
                      nc.sync.dma_start(c_[:], w2t.ap()[j, ts(i_, 128), :])
                      w2ch.append(c_)
                  for t in range(TT):
                      for h in range(4):
                          ps_d = pdn.tile([128, 512], F32, tag="dn")
                          for i_ in range(IT):
                              nc.tensor.matmul(ps_d[:],
                                               actw[j][i_][:, ts(t, 128)],
                                               w2ch[i_][:, ds(h * 512, 512)],
                                               start=(i_ == 0),
                                               stop=(i_ == IT - 1))
                          sl = acc[t][:, ds(h * 512, 512)]
                          nc.vector.tensor_add(sl, sl, ps_d[:])

              # ---------- combine across cores ----------
              for t in range(TT):
                  nc.sync.dma_start(cc_in.ap()[ts(t, 128), :], acc[t][:])
              nc.gpsimd.collective_compute(
                  "ReduceScatter", ALU.add,
                  replica_groups=[list(range(N_CORES))],
                  ins=[cc_in.ap().opt()], outs=[cc_out.ap().opt()])
              nc.sync.dma_start(y.ap(), cc_out.ap())

    nc.compile()
    return nc


# ---------------------------------------------------------------------------
# host-side sharding / layout prep
# ---------------------------------------------------------------------------

def prep_in_maps(hidden_states, gate_w, gate_bias, sh_wg, sh_wu, sh_wd,
                 ex_w1, ex_w3, ex_w2):
    bf = ml_dtypes.bfloat16
    x = np.ascontiguousarray(np.asarray(hidden_states, np.float32))
    xt = np.ascontiguousarray(x.T)                     # [H, T] f32
    xt_bf = xt.astype(bf)
    gwt = np.ascontiguousarray(np.asarray(gate_w, np.float32).T)  # [H, E]
    gb = np.ascontiguousarray(np.asarray(gate_bias, np.float32).reshape(1, E))

    w1t = np.ascontiguousarray(
        np.asarray(ex_w1, np.float32).transpose(0, 2, 1)).astype(bf)  # [E,H,I]
    w3t = np.ascontiguousarray(
        np.asarray(ex_w3, np.float32).transpose(0, 2, 1)).astype(bf)
    w2t_full = np.ascontiguousarray(
        np.asarray(ex_w2, np.float32).transpose(0, 2, 1)).astype(bf)  # [E,I,H]
    shwgt = np.ascontiguousarray(np.asarray(sh_wg, np.float32).T).astype(bf)
    shwut = np.ascontiguousarray(np.asarray(sh_wu, np.float32).T).astype(bf)
    shwdt_full = np.ascontiguousarray(
        np.asarray(sh_wd, np.float32).T).astype(bf)

    in_maps = []
    for c in range(N_CORES):
        sl = slice(c * E_LOC, (c + 1) * E_LOC)
        sel = np.zeros((E_LOC, E, 128), np.float32)
        for j in range(E_LOC):
            sel[j, c * E_LOC + j, :] = 1.0
        w13 = np.stack([w1t[sl], w3t[sl]], axis=1)     # [E_LOC, 2, H, I]
        ssl = slice(c * SH, (c + 1) * SH)
        in_maps.append({
            "xt_bf": xt_bf,
            "xt_f32": xt,
            "gwt": gwt,
            "gbias": gb,
            "selb": sel,
            "w13t": np.ascontiguousarray(w13),
            "w2t": np.ascontiguousarray(w2t_full[sl]),
            "shgut": np.ascontiguousarray(
                np.stack([shwgt[:, ssl], shwut[:, ssl]], axis=0)),
            "shwdt": np.ascontiguousarray(shwdt_full[ssl, :]),
        })
    return in_maps


_CACHE = {}


def get_nc():
    if "nc" not in _CACHE:
        _CACHE["nc"] = build_nc()
    return _CACHE["nc"]


def kernel(**inputs) -> np.ndarray:
    nc = get_nc()
    in_maps = prep_in_maps(**inputs)
    from concourse.bass_utils import run_bass_kernel_spmd
    res = run_bass_kernel_spmd(nc, in_maps, core_ids=list(range(N_CORES)))
    out = np.concatenate([res.results[c]["y"] for c in range(N_CORES)], axis=0)
    return np.ascontiguousarray(out.astype(np.float32))

